# revision 55
# baseline (speedup 1.0000x reference)
"""Trainium2 Bass kernel for nn_DFlashDecoderLayer (dense transformer decoder layer:
self-attn + cross-attn + SwiGLU MLP, B=1, S=2048, H=1024, NH=16, HD=64, I=4096).

Sharding strategy (8 NeuronCores, SPMD):
  Sequence-sharded: core r owns query rows [256r, 256r+256).  Every matmul weight is
  used in full by every core, in bf16.  The only cross-core data dependency is full-
  sequence K/V for the two attention blocks; each core computes K/V for its own rows
  (all heads) and a single AllGather per K/V matrix shares them.  Norm weights +
  1/sqrt(HD) score scale are folded into the following projection weights host-side;
  the rmsnorm 1/rstd column scale is folded into the PSUM->SBUF copies of q/k/v so
  the K/V matmuls run on RAW x (bf16 copy shipped from host) without waiting for the
  norm — this launches the first AllGather as early as possible.

  Queue discipline (the schedule lives or dies on this):
   - sync (SP) queue: weight slab loads ONLY (no data deps -> continuous stream,
     deep MLP prefetch), plus the final output store.
   - scalar (Activation) queue: input loads, K/V staging writes, kf/vsb table
     loads, and the scalar compute (Square/Exp/Silu).  Everything here is in
     natural pipeline order with its own gates.
   - gpsimd queue: the four collective_computes ONLY.  A collective blocks its
     queue until the previous collective COMPLETES on the CC engine, so nothing
     else may live there.

  On-chip layout is feature-major ("transposed"): activations live as [H_part, seq]
  so weight matrices ([in, out] row-major) serve directly as matmul lhsT tiles and
  layer outputs chain without transposes.  Scores are computed transposed [k, q];
  the softmax denominator comes from a ones-column augmented onto V (M=65 matmul),
  and the 1/l division uses a K=1 ones outer-product matmul as the partition
  broadcast (gpsimd partition_broadcast / remote_dma do not compile on this walrus
  build).  The AV epilogue (reciprocal -> broadcast matmul -> scale) is software-
  pipelined one head-pair behind the AV matmuls so the PE never waits on the DVE
  chain.

  PE p-state: the clock only ramps 1.2->2.4GHz after ~3us of uninterrupted
  streaming, and per-matmul start/stop group boundaries reset it.  All
  projection/MLP matmuls therefore run as 32-matmul interleaved streams: two
  psum banks, each holding an m-pair in its 256-col halves (start=True clears
  the whole bank; first write to each half lands on has_written=0), with one
  start/stop per bank.  AV runs both heads of a pair as one 32-matmul group.
"""

import os
import sys

sys.path.insert(0, "/opt/trn_rl_repo")

import numpy as np
import ml_dtypes

import concourse.bass as bass
import concourse.mybir as mybir
import concourse.tile as tile

H = 1024      # hidden size
S = 2048      # sequence length
NH = 16       # heads
HD = 64       # head dim
I = 4096      # mlp intermediate
NC = 8        # cores
R = S // NC   # rows per core = 256
HT = H // 128  # hidden tiles = 8
KT = S // 128  # key tiles = 16
EPS = 1e-6

F32 = mybir.dt.float32
BF16 = mybir.dt.bfloat16
AF = mybir.ActivationFunctionType
BF16NP = ml_dtypes.bfloat16

_CACHED_MODULE = None


def _split_multi_waits(nc):
    """This env's walrus rejects >1 sem wait per instruction.
    Hoist extra waits onto preceding single-wait NoOps on the same engine."""
    limit = 1
    n_split = 0
    for f in nc.m.functions:
        for bb in f.blocks:
            new_insts = []
            for inst in bb.instructions:
                si = getattr(inst, "sync_info", None)
                if (si is not None and getattr(inst, "engine", None) is not None
                        and len(si.on_wait) > limit):
                    waits = list(si.on_wait)
                    hoist, keep = waits[:-limit], waits[-limit:]
                    for i, w in enumerate(hoist):
                        new_insts.append(
                            mybir.InstNoOp(
                                name=f"{inst.name}_waitsplit_{i}",
                                engine=inst.engine,
                                sync_info=mybir.SyncInfo(on_wait=[w], on_update=[]),
                                bass_nofuse=True,
                            )
                        )
                        n_split += 1
                    si.on_wait = keep
                new_insts.append(inst)
            bb.instructions = new_insts
    return n_split


def build_module():
    global _CACHED_MODULE
    if _CACHED_MODULE is not None:
        return _CACHED_MODULE

    nc = bass.Bass(num_devices=NC)

    # --- kernel I/O (per-core) ---
    xT = nc.declare_dram_parameter("xT", [H, R], F32, isOutput=False)
    xbT = nc.declare_dram_parameter("xbT", [H, R], BF16, isOutput=False)
    ctxT = nc.declare_dram_parameter("ctxT", [H, R], BF16, isOutput=False)
    wnames = ["sa_wq", "sa_wk", "sa_wv", "sa_wo", "ca_wq", "ca_wk", "ca_wv", "ca_wo"]
    W = {n: nc.declare_dram_parameter(n, [H, H], BF16, isOutput=False) for n in wnames}
    W["w_gate"] = nc.declare_dram_parameter("w_gate", [H, I], BF16, isOutput=False)
    W["w_up"] = nc.declare_dram_parameter("w_up", [H, I], BF16, isOutput=False)
    W["w_down"] = nc.declare_dram_parameter("w_down", [I, H], BF16, isOutput=False)
    outT = nc.declare_dram_parameter("outT", [H, R], F32, isOutput=True)

    groups = [list(range(NC))]

    with tile.TileContext(nc) as tc:
        with (
            tc.tile_pool(name="p1", bufs=1) as p1,        # long-lived singles
            tc.tile_pool(name="p2", bufs=2) as p2,        # rotating pairs
            tc.tile_pool(name="resid", bufs=2) as presid, # xT / h1 / h2 fp32
            tc.tile_pool(name="wts", bufs=4) as pw,       # 2MB weight slabs
            tc.tile_pool(name="psA", bufs=5, space="PSUM") as psA,
            tc.tile_pool(name="psB", bufs=3, space="PSUM") as psB,
            tc.tile_pool(name="dram", bufs=1, space="DRAM") as pdram,
        ):
            # --- constants ---
            inv_h = p1.tile([128, 1], BF16, tag="inv_h")
            nc.vector.memset(inv_h[:], 1.0 / H)
            eps_c = p1.tile([1, 1], F32, tag="eps_c")
            nc.vector.memset(eps_c[:], EPS)
            # ones row for K=1 outer-product broadcasts
            ones_row = p1.tile([1, 128], BF16, tag="ones_row")
            nc.vector.memset(ones_row[:], 1.0)
            # --- input loads: xb/xt FIRST on sync (ahead of the weight stream —
            # they gate the whole front of the schedule); ctx on scalar (needed
            # ~40us later, rides the leftover bandwidth) ---
            xb_sb = p1.tile([128, HT, R], BF16, tag="xb_sb")
            nc.sync.dma_start(xb_sb[:], xbT.rearrange("(t p) q -> p t q", p=128))
            xt_sb = presid.tile([128, HT, R], F32, tag="resid", name="xt_sb")
            nc.sync.dma_start(xt_sb[:], xT.rearrange("(t p) q -> p t q", p=128))
            ctx_sb = p1.tile([128, HT, R], BF16, tag="ctx_sb")
            nc.sync.dma_start(ctx_sb[:], ctxT.rearrange("(t p) q -> p t q", p=128))

            def load_w(dram_t, cols=None, rows=None, name="w"):
                """Load a [1024, M<=1024] slab of a weight matrix as [128, 8, M] bf16
                on the sync queue (weights only -> continuous prefetch stream)."""
                ap = dram_t.rearrange("(t p) m -> p t m", p=128)
                if rows is not None:  # row-chunk of a tall matrix (w_down)
                    ap = dram_t[rows[0]:rows[1], :].rearrange("(t p) m -> p t m", p=128)
                if cols is not None:
                    ap = ap[:, :, cols[0]:cols[1]]
                m = ap.shape[2]
                t = pw.tile([128, HT, 1024], BF16, tag="w", name=name)
                nc.sync.dma_start(t[:, :, :m], ap)
                return t

            def rstd_broadcast(src_f32, name, pool=None):
                """1/rms of each column of src, broadcast to a [128, R] bf16 SBUF
                tile (partition-broadcast via K=1 ones outer-product matmul)."""
                pool = pool or psA
                var = pool.tile([128, 512], F32, tag=pool is psA and "psA" or "psB",
                                name=f"{name}_var")
                for t in range(HT):
                    sq = p2.tile([128, R], BF16, tag="sq", name=f"{name}_sq{t}")
                    nc.scalar.activation(sq[:], src_f32[:, t, :], AF.Square)
                    nc.tensor.matmul(var[:1, :R], inv_h[:], sq[:],
                                     start=(t == 0), stop=(t == HT - 1))
                sd = p2.tile([1, R], F32, tag="sd", name=f"{name}_sd")
                nc.scalar.activation(sd[:], var[:1, :R], AF.Sqrt, bias=eps_c[:])
                rstd = p2.tile([1, R], BF16, tag="rstd", name=f"{name}_rstd")
                with nc.allow_low_precision(reason="bf16 rstd broadcast, tol 2e-2"):
                    nc.vector.reciprocal(rstd[:], sd[:])
                rb = pool.tile([128, 512], F32, tag=pool is psA and "psA" or "psB",
                               name=f"{name}_rb")
                # both 256-halves get the same broadcast so 512-wide consumers
                # (m-paired psum tiles) can multiply in one op
                nc.tensor.matmul(rb[:, :R], ones_row[:1, :], rstd[:1, :],
                                 start=True, stop=False)
                nc.tensor.matmul(rb[:, R:2 * R], ones_row[:1, :], rstd[:1, :],
                                 start=False, stop=True, skip_group_check=True)
                rbs = p1.tile([128, 2 * R], BF16, tag=f"rbs_{name}", name=f"{name}_rbs")
                nc.vector.tensor_copy(rbs[:], rb[:, :2 * R])
                return rbs

            def mm_pair(ps, w_sb, mp, act_sb):
                """16-matmul contiguous accumulation: output tiles 2mp/2mp+1
                side by side in one psum bank.  start=True on the first clears
                the whole bank; the first write to each 256-col half lands on
                has_written=0 so accumulates from zero.  Long contiguous groups
                keep the PE p-state ramped."""
                for t in range(HT):
                    for half in range(2):
                        m = 2 * mp + half
                        nc.tensor.matmul(
                            ps[:, R * half:R * (half + 1)],
                            w_sb[:, t, 128 * m:128 * (m + 1)], act_sb[:, t, :],
                            start=(t == 0 and half == 0),
                            stop=(t == HT - 1 and half == 1),
                            skip_group_check=True)

            def mm_quad(specs, act_sb):
                """Two m-pair accumulations (separate psum banks) interleaved
                into ONE 32-matmul PE stream with a single start/stop per bank.
                32-long uninterrupted streams are what actually reach the
                2.4GHz p-state (16-long groups stay at 1.2GHz).
                specs: [(ps, w_sb, mp), (ps2, w_sb2, mp2)]."""
                for t in range(HT):
                    for half in range(2):
                        for ps, w_sb, mp in specs:
                            m = 2 * mp + half
                            nc.tensor.matmul(
                                ps[:, R * half:R * (half + 1)],
                                w_sb[:, t, 128 * m:128 * (m + 1)], act_sb[:, t, :],
                                start=(t == 0 and half == 0),
                                stop=(t == HT - 1 and half == 1),
                                skip_group_check=True)

            def proj(w_sb, act_sb, dst_bf16, rbs=None):
                """dst[*, m, :] (bf16 [128, HT, R]) = (W.T @ act) [* rbs column scale]."""
                for mq in range(HT // 4):
                    pss = [psA.tile([128, 512], F32, tag="psA", name=f"pj_{2 * mq + i}")
                           for i in range(2)]
                    mm_quad([(pss[0], w_sb, 2 * mq), (pss[1], w_sb, 2 * mq + 1)], act_sb)
                    for i in range(2):
                        mp = 2 * mq + i
                        dst2 = dst_bf16[:, 2 * mp:2 * mp + 2, :].rearrange("p a b -> p (a b)")
                        if rbs is None:
                            nc.vector.tensor_copy(dst2, pss[i][:, :2 * R])
                        else:
                            nc.vector.tensor_mul(dst2, pss[i][:, :2 * R], rbs[:])

            def proj_add(w_sb, act_sb, resid_f32, dst_f32):
                """dst (f32 [128, HT, R]) = resid + W.T @ act."""
                for mq in range(HT // 4):
                    pss = [psA.tile([128, 512], F32, tag="psA", name=f"pa_{2 * mq + i}")
                           for i in range(2)]
                    mm_quad([(pss[0], w_sb, 2 * mq), (pss[1], w_sb, 2 * mq + 1)], act_sb)
                    for i in range(2):
                        mp = 2 * mq + i
                        nc.vector.tensor_add(
                            dst_f32[:, 2 * mp:2 * mp + 2, :].rearrange("p a b -> p (a b)"),
                            pss[i][:, :2 * R],
                            resid_f32[:, 2 * mp:2 * mp + 2, :].rearrange("p a b -> p (a b)"))

            def kv_block(wk_sb, wv_sb, act_k, act_v_fn, blk,
                         rstd_src=None, rstd_name=None):
                """Compute own-row K^T [1024, R] and V [R, 1024] (bf16), stage to
                DRAM (scalar queue) and AllGather (gpsimd).  K runs on act_k (raw
                activation); if rstd_src is given, the rmsnorm rstd is computed
                CONCURRENTLY (var/rb on psB, interleaved after K group 3 so the
                PE starts on K immediately) and folded into the PSUM copies.
                act_v_fn(rbs) supplies the V activation (pre-normed; V's gather
                has slack behind K's).  Returns (k_out, v_out, rbs)."""
                k_in = pdram.tile([H * R], BF16, tag=f"kin{blk}")
                k_out = pdram.tile([NC, H * R], BF16, tag=f"kout{blk}",
                                   addr_space="Shared")
                v_in = pdram.tile([H * R], BF16, tag=f"vin{blk}")
                v_out = pdram.tile([NC, H * R], BF16, tag=f"vout{blk}",
                                   addr_space="Shared")
                k_view = k_in.rearrange("(t p q) -> p t q", t=HT, p=128, q=R)
                v_view = v_in.rearrange("(mt p d) -> p mt d", mt=2, p=128, d=1024)
                rbs = None
                pending = []

                def emit_k_copy(mp, ps):
                    stg = p2.tile([128, 512], BF16, tag="stg", bufs=2,
                                  name=f"ks{blk}_{mp}")
                    if rbs is None:
                        nc.vector.tensor_copy(stg[:, :2 * R], ps[:, :2 * R])
                    else:
                        nc.vector.tensor_mul(stg[:, :2 * R], ps[:, :2 * R], rbs[:])
                    nc.scalar.dma_start(
                        k_view[:, 2 * mp:2 * mp + 2, :],
                        stg[:, :2 * R].rearrange("p (a b) -> p a b", a=2))

                for mq in range(HT // 4):
                    pss = [psA.tile([128, 512], F32, tag="psA", name=f"k{blk}_{2 * mq + i}")
                           for i in range(2)]
                    mm_quad([(pss[0], wk_sb, 2 * mq), (pss[1], wk_sb, 2 * mq + 1)], act_k)
                    if rstd_src is not None and rbs is None:
                        pending += [(2 * mq, pss[0]), (2 * mq + 1, pss[1])]
                        if mq == 0:
                            rbs = rstd_broadcast(rstd_src, rstd_name, pool=psB)
                            for mmp, pps in pending:
                                emit_k_copy(mmp, pps)
                            pending = []
                    else:
                        for i in range(2):
                            emit_k_copy(2 * mq + i, pss[i])
                nc.gpsimd.collective_compute(
                    "AllGather", mybir.AluOpType.bypass, replica_groups=groups,
                    ins=[k_in[:]], outs=[k_out[:]])
                act_v = act_v_fn(rbs)
                for mt in range(2):
                    for nchunk in range(2):
                        ps = psA.tile([128, 512], F32, tag="psA", name=f"v{blk}_{mt}_{nchunk}")
                        for t in range(HT):
                            nc.tensor.matmul(
                                ps[:], act_v[:, t, 128 * mt:128 * (mt + 1)],
                                wv_sb[:, t, 512 * nchunk:512 * (nchunk + 1)],
                                start=(t == 0), stop=(t == HT - 1))
                        stg = p2.tile([128, 512], BF16, tag="stg", bufs=2,
                                      name=f"vs{blk}_{mt}_{nchunk}")
                        nc.vector.tensor_copy(stg[:], ps[:])
                        nc.scalar.dma_start(v_view[:, mt, 512 * nchunk:512 * (nchunk + 1)], stg[:])
                nc.gpsimd.collective_compute(
                    "AllGather", mybir.AluOpType.bypass, replica_groups=groups,
                    ins=[v_in[:]], outs=[v_out[:]])
                return k_out, v_out, rbs

            def attention(q_sb, k_out, v_out, blk):
                """q_sb [128, HT, R] bf16 (feature-major, all heads), k_out/v_out from
                kv_block. Returns attnT [128, HT, R] bf16 = softmax(qk)V transposed."""
                vsb = p1.tile([128, KT, NH, HD + 1], BF16, tag="vsb", name=f"vsb{blk}")
                attnT = p1.tile([128, HT, R], BF16, tag="attnT", name=f"attnT{blk}")
                nc.vector.memset(vsb[:, :, :, HD:HD + 1], 1.0)

                def load_vsb():
                    # sync queue: gated on the V AllGather.  On scalar it would
                    # head-of-line block the kf/EXP stream; on sync it only
                    # pauses weight prefetch, which has slack here.  Emitted at
                    # the END of dt==0 so every av_pair (first at dt==1) is
                    # emission-ordered after these writes — Tile only orders
                    # reads after writes already emitted.  Two batched triggers
                    # (ranks 0-3 / 4-7) instead of 16 per-kt ones.
                    for kt in range(KT):
                        r, mt = kt // 2, kt % 2
                        src = v_out[r].rearrange(
                            "(mt p hd d) -> p mt hd d", mt=2, p=128, hd=NH, d=HD)
                        nc.sync.dma_start(vsb[:, kt, :, 0:HD], src[:, mt, :, :])

                pts_all = []
                for dt in range(HT):  # head pair dt = heads 2dt, 2dt+1
                    kf = p2.tile([128, NC, R], BF16, tag="kf", bufs=2, name=f"kf{blk}_{dt}")
                    # ONE batched trigger for all 8 ranks, on the SYNC queue:
                    # the scalar sequencer must stay pure EXP during attention
                    # (EXP throughput is the phase's floor).
                    nc.sync.dma_start(
                        kf[:, :, :],
                        k_out.rearrange("r (t p q) -> p t r q", t=HT, p=128, q=R)[:, dt, :, :])
                    # Both heads' score matmuls are emitted ADJACENTLY so the
                    # (0,0)/(64,0) tile_position row-packing runs them
                    # concurrently on the PE array.
                    # bufs=4: the lag-1 AV keeps pair dt-1 alive while pair dt's
                    # EXPs write.  MUST be >= 4 — with 3, pt(dt,1) would steal
                    # pt(dt-1,0)'s slot before its AV reads are even emitted.
                    pts = [p2.tile([128, KT, R], BF16, tag="pt", bufs=4,
                                   name=f"pt{blk}_{2 * dt + hh}") for hh in range(2)]
                    for kp in range(KT // 2):
                        spss = [psA.tile([128, 512], F32, tag="psA",
                                         name=f"s{blk}_{2 * dt + hh}_{kp}") for hh in range(2)]
                        for j in range(2):
                            kt = 2 * kp + j
                            r2, mt2 = kt // 2, kt % 2
                            for hh in range(2):
                                off = HD * hh
                                # one start/stop per psum tile (j halves share a
                                # group) — per-matmul start/stop resets the PE
                                # p-state ramp
                                nc.tensor.matmul(
                                    spss[hh][:, R * j:R * (j + 1)],
                                    kf[off:off + HD, r2, 128 * mt2:128 * (mt2 + 1)],
                                    q_sb[off:off + HD, dt, :],
                                    start=(j == 0), stop=(j == 1),
                                    tile_position=(off, 0), skip_group_check=True)
                        for hh in range(2):
                            nc.scalar.activation(
                                pts[hh][:, 2 * kp:2 * kp + 2, :],
                                spss[hh].rearrange("p (a b) -> p a b", a=2), AF.Exp)
                    pts_all.append(pts)
                    if dt == 0:
                        load_vsb()

                    # AV for head pair dt-1 (lag so scores of dt overlap the
                    # vsb wait / epilogue of earlier heads)
                    if dt >= 1:
                        av_pair(vsb, attnT, pts_all[dt - 1], dt - 1, blk)
                av_pair(vsb, attnT, pts_all[HT - 1], HT - 1, blk)
                flush_epilogue(attnT, blk)
                return attnT

            # --- AV + epilogue with software pipelining (module-level state) ---
            epi_state = []

            def av_pair(vsb, attnT, pts, dt, blk):
                """Both heads of pair dt in ONE 32-matmul psum group (heads side
                by side in the bank's 256-col halves) — long contiguous PE runs
                keep the p-state ramped."""
                avps = psB.tile([128, 512], F32, tag="psB", name=f"av{blk}_{dt}")
                for kt in range(KT):
                    for hh in range(2):
                        nc.tensor.matmul(
                            avps[:HD + 1, R * hh:R * (hh + 1)],
                            vsb[:, kt, 2 * dt + hh, :], pts[hh][:, kt, :],
                            start=(kt == 0 and hh == 0),
                            stop=(kt == KT - 1 and hh == 1),
                            skip_group_check=True)
                # vector part of the pair's epilogue: one 512-wide recip + copy
                rl = p2.tile([1, 2 * R], BF16, tag="rl", name=f"rl{blk}_{dt}")
                with nc.allow_low_precision(reason="bf16 1/l broadcast, tol 2e-2"):
                    nc.vector.reciprocal(rl[:], avps[HD:HD + 1, :2 * R])
                av_sb = p2.tile([HD, 2 * R], BF16, tag="av_sb", bufs=2,
                                name=f"avs{blk}_{dt}")
                nc.vector.tensor_copy(av_sb[:], avps[0:HD, :2 * R])
                epi_state.append((rl, av_sb, dt))
                # PE part (rlb broadcast matmuls) of the OLDEST pending pair:
                # one pair behind, so its wait on the DVE reciprocal is hidden
                # behind this pair's AV stream.
                if len(epi_state) > 1:
                    emit_rlb(attnT, blk, epi_state.pop(0))

            def emit_rlb(attnT, blk, st):
                rl, av_sb, dt = st
                rlb = psB.tile([128, 512], F32, tag="psB", name=f"rlb{blk}_{dt}")
                nc.tensor.matmul(rlb[:HD, :R], ones_row[:1, :HD], rl[:1, :R],
                                 start=True, stop=False)
                nc.tensor.matmul(rlb[:HD, R:2 * R], ones_row[:1, :HD], rl[:1, R:2 * R],
                                 start=False, stop=True, skip_group_check=True)
                for hh in range(2):
                    off = HD * hh
                    nc.vector.tensor_mul(attnT[off:off + HD, dt, :],
                                         av_sb[:, R * hh:R * (hh + 1)],
                                         rlb[:HD, R * hh:R * (hh + 1)])

            def flush_epilogue(attnT, blk):
                while epi_state:
                    emit_rlb(attnT, blk, epi_state.pop(0))

            # ---------------- self-attention block ----------------
            wk_sb = load_w(W["sa_wk"], name="sa_wk_sb")
            wv_sb = load_w(W["sa_wv"], name="sa_wv_sb")

            def make_xn(rbs):
                # normed x for the V projection
                xn = p1.tile([128, HT, R], BF16, tag="normed", name="xn")
                for t in range(HT):
                    nc.vector.tensor_mul(xn[:, t, :], xb_sb[:, t, :], rbs[:, :R])
                return xn

            k1, v1, rbs0 = kv_block(wk_sb, wv_sb, xb_sb, make_xn, 0,
                                    rstd_src=xt_sb, rstd_name="n0")

            # cross-attn K/V depend only on raw context: compute + AG them early
            # so both gathers overlap the self-attention phase on the CC engine.
            wk2_sb = load_w(W["ca_wk"], name="ca_wk_sb")
            wv2_sb = load_w(W["ca_wv"], name="ca_wv_sb")
            k2, v2, _ = kv_block(wk2_sb, wv2_sb, ctx_sb, lambda rbs: ctx_sb, 1)

            # Weight-load EMISSION order is chosen so each pw-slot reuse is
            # emitted after the previous occupant's readers (Tile only orders
            # against already-emitted reads), and so every slab's runtime gate
            # (slot free) opens just before the data is needed:
            #   slabs 1-8: wk,wv,wk2,wv2,wq | wo,wq2,wo2   9-12: wg0,wu0 wg1,wu1
            #   13-16: wg2,wu2,wg3,wu3 (emitted inside the MLP loop)
            #   17-20: wd0-3 (emitted inside the MLP loop)
            wq_sb = load_w(W["sa_wq"], name="sa_wq_sb")
            wo_sb = load_w(W["sa_wo"], name="sa_wo_sb")
            wq2_sb = load_w(W["ca_wq"], name="ca_wq_sb")
            wo2_sb = load_w(W["ca_wo"], name="ca_wo_sb")
            qT = p1.tile([128, HT, R], BF16, tag="qt", name="qT")
            proj(wq_sb, xb_sb, qT, rbs=rbs0)
            wgu = [load_w(W["w_gate"], cols=(0, 1024), name="wg0")]

            attnT = attention(qT, k1, v1, 0)
            h1 = presid.tile([128, HT, R], F32, tag="resid", name="h1")
            proj_add(wo_sb, attnT, xt_sb, h1)
            wgu.append(load_w(W["w_up"], cols=(0, 1024), name="wu0"))

            # ---------------- cross-attention block ----------------
            rbs1 = rstd_broadcast(h1, "n1")
            h1b = p1.tile([128, HT, R], BF16, tag="normed", name="h1b")
            for t in range(HT):
                nc.vector.tensor_copy(h1b[:, t, :], h1[:, t, :])
            qT2 = p1.tile([128, HT, R], BF16, tag="qt", name="qT2")
            proj(wq2_sb, h1b, qT2, rbs=rbs1)
            wgu.append(load_w(W["w_gate"], cols=(1024, 2048), name="wg1"))

            attnT2 = attention(qT2, k2, v2, 1)
            h2 = presid.tile([128, HT, R], F32, tag="resid", name="h2")
            proj_add(wo2_sb, attnT2, h1, h2)
            wgu.append(load_w(W["w_up"], cols=(1024, 2048), name="wu1"))

            # ---------------- MLP block ----------------
            # NOTE: start=True clears has_written for the WHOLE psum bank, so each
            # accumulation group must own its bank exclusively for its entire
            # lifetime.  Phase A computes all 32 act subtiles into SBUF; phase B
            # runs one contiguous 32-matmul accumulation per output tile.
            rbs2 = rstd_broadcast(h2, "n2")
            hn2 = p1.tile([128, HT, R], BF16, tag="normed", name="hn2")
            for t in range(HT):
                nc.vector.tensor_mul(hn2[:, t, :], h2[:, t, :], rbs2[:, :R])
            NCHUNK = 4  # I-chunks of 1024
            # reuses xb_sb's slot (its readers — K1/q matmuls — are long done)
            act_full = p1.tile([128, I // 128, R], BF16, tag="xb_sb")  # 2MB
            wds = []
            for c in range(NCHUNK):
                wg_sb, wu_sb = wgu[2 * c], wgu[2 * c + 1]
                for mip in range(4):
                    gps = psA.tile([128, 512], F32, tag="psA", name=f"g{c}_{mip}")
                    ups = psA.tile([128, 512], F32, tag="psA", name=f"u{c}_{mip}")
                    mm_quad([(gps, wg_sb, mip), (ups, wu_sb, mip)], hn2)
                    gsil = p2.tile([128, 2 * R], BF16, tag="gsil", name=f"gs{c}_{mip}")
                    nc.scalar.activation(gsil[:], gps[:, :2 * R], AF.Silu)
                    nc.vector.tensor_mul(
                        act_full[:, 8 * c + 2 * mip:8 * c + 2 * mip + 2, :]
                        .rearrange("p a b -> p (a b)"),
                        ups[:, :2 * R], gsil[:])
                # prefetch: next gate/up pair, then the down slabs
                if c < NCHUNK - 2:
                    cc = c + 2
                    wgu.append(load_w(W["w_gate"], cols=(1024 * cc, 1024 * (cc + 1)), name=f"wg{cc}"))
                    wgu.append(load_w(W["w_up"], cols=(1024 * cc, 1024 * (cc + 1)), name=f"wu{cc}"))
                elif c == NCHUNK - 2:
                    wds.append(load_w(W["w_down"], rows=(0, 1024), name="wd0"))
                    wds.append(load_w(W["w_down"], rows=(1024, 2048), name="wd1"))
                else:
                    wds.append(load_w(W["w_down"], rows=(2048, 3072), name="wd2"))
                    wds.append(load_w(W["w_down"], rows=(3072, 4096), name="wd3"))
            out_sb = p1.tile([128, HT, R], F32, tag="out_sb")
            for m in range(HT):
                dps = psB.tile([128, 512], F32, tag="psB", name=f"dp{m}")
                for s in range(I // 128):
                    nc.tensor.matmul(dps[:, :R], wds[s // 8][:, s % 8, 128 * m:128 * (m + 1)],
                                     act_full[:, s, :],
                                     start=(s == 0), stop=(s == I // 128 - 1))
                nc.vector.tensor_add(out_sb[:, m, :], dps[:, :R], h2[:, m, :])
            nc.sync.dma_start(outT.rearrange("(t p) q -> p t q", p=128), out_sb[:])

    _split_multi_waits(nc)
    _CACHED_MODULE = nc
    return nc


def prep_in_maps(hidden_states, context, sa_norm_w, sa_wq, sa_wk, sa_wv, sa_wo,
                 ca_norm_w, ca_wq, ca_wk, ca_wv, ca_wo,
                 mlp_norm_w, w_gate, w_up, w_down):
    f32 = np.float32
    x = np.asarray(hidden_states, f32).reshape(S, H)
    ctx = np.asarray(context, f32).reshape(S, H)
    xT_full = np.ascontiguousarray(x.T)                      # [H, S] f32
    xbT_full = xT_full.astype(BF16NP)                        # [H, S] bf16
    ctxT_full = np.ascontiguousarray(ctx.T).astype(BF16NP)   # [H, S] bf16

    def bf(a):
        return np.ascontiguousarray(np.asarray(a, f32)).astype(BF16NP)

    sa_w = np.asarray(sa_norm_w, f32)[:, None]
    ca_w = np.asarray(ca_norm_w, f32)[:, None]
    mlp_w = np.asarray(mlp_norm_w, f32)[:, None]
    scale = HD ** -0.5
    shared = {
        "sa_wq": bf(sa_w * np.asarray(sa_wq, f32) * scale),
        "sa_wk": bf(sa_w * np.asarray(sa_wk, f32)),
        "sa_wv": bf(sa_w * np.asarray(sa_wv, f32)),
        "sa_wo": bf(sa_wo),
        "ca_wq": bf(ca_w * np.asarray(ca_wq, f32) * scale),
        "ca_wk": bf(ca_wk),
        "ca_wv": bf(ca_wv),
        "ca_wo": bf(ca_wo),
        "w_gate": bf(mlp_w * np.asarray(w_gate, f32)),
        "w_up": bf(mlp_w * np.asarray(w_up, f32)),
        "w_down": bf(w_down),
    }
    in_maps = []
    for r in range(NC):
        m = dict(shared)
        m["xT"] = np.ascontiguousarray(xT_full[:, r * R:(r + 1) * R])
        m["xbT"] = np.ascontiguousarray(xbT_full[:, r * R:(r + 1) * R])
        m["ctxT"] = np.ascontiguousarray(ctxT_full[:, r * R:(r + 1) * R])
        in_maps.append(m)
    return in_maps


def run_spmd(in_maps, **kwargs):
    from concourse.bass_utils import run_bass_kernel_spmd
    nc = build_module()
    return run_bass_kernel_spmd(nc, in_maps, core_ids=list(range(NC)), **kwargs)


def kernel(**inputs):
    in_maps = prep_in_maps(**inputs)
    res = run_spmd(in_maps)
    out = np.empty((1, S, H), np.float32)
    for r in range(NC):
        out[0, r * R:(r + 1) * R, :] = res.results[r]["outT"].T
    return out


# revision 56
# speedup vs baseline: 1.0124x; 1.0124x over previous
"""Trainium2 Bass kernel for nn_DFlashDecoderLayer (dense transformer decoder layer:
self-attn + cross-attn + SwiGLU MLP, B=1, S=2048, H=1024, NH=16, HD=64, I=4096).

Sharding strategy (8 NeuronCores, SPMD):
  Sequence-sharded: core r owns query rows [256r, 256r+256).  Every matmul weight is
  used in full by every core, in bf16.  The only cross-core data dependency is full-
  sequence K/V for the two attention blocks; each core computes K/V for its own rows
  (all heads) and a single AllGather per K/V matrix shares them.  Norm weights +
  1/sqrt(HD) score scale are folded into the following projection weights host-side;
  the rmsnorm 1/rstd column scale is folded into the PSUM->SBUF copies of q/k/v so
  the K/V matmuls run on RAW x (bf16 copy shipped from host) without waiting for the
  norm — this launches the first AllGather as early as possible.

  Queue discipline (the schedule lives or dies on this):
   - sync (SP) queue: weight slab loads ONLY (no data deps -> continuous stream,
     deep MLP prefetch), plus the final output store.
   - scalar (Activation) queue: input loads, K/V staging writes, kf/vsb table
     loads, and the scalar compute (Square/Exp/Silu).  Everything here is in
     natural pipeline order with its own gates.
   - gpsimd queue: the four collective_computes ONLY.  A collective blocks its
     queue until the previous collective COMPLETES on the CC engine, so nothing
     else may live there.

  On-chip layout is feature-major ("transposed"): activations live as [H_part, seq]
  so weight matrices ([in, out] row-major) serve directly as matmul lhsT tiles and
  layer outputs chain without transposes.  Scores are computed transposed [k, q];
  the softmax denominator comes from a ones-column augmented onto V (M=65 matmul),
  and the 1/l division uses a K=1 ones outer-product matmul as the partition
  broadcast (gpsimd partition_broadcast / remote_dma do not compile on this walrus
  build).  The AV epilogue (reciprocal -> broadcast matmul -> scale) is software-
  pipelined one head-pair behind the AV matmuls so the PE never waits on the DVE
  chain.

  PE p-state: the clock only ramps 1.2->2.4GHz after ~3us of uninterrupted
  streaming, and per-matmul start/stop group boundaries reset it.  All
  projection/MLP matmuls therefore run as 32-matmul interleaved streams: two
  psum banks, each holding an m-pair in its 256-col halves (start=True clears
  the whole bank; first write to each half lands on has_written=0), with one
  start/stop per bank.  AV runs both heads of a pair as one 32-matmul group.
"""

import os
import sys

sys.path.insert(0, "/opt/trn_rl_repo")

import numpy as np
import ml_dtypes

import concourse.bass as bass
import concourse.mybir as mybir
import concourse.tile as tile

H = 1024      # hidden size
S = 2048      # sequence length
NH = 16       # heads
HD = 64       # head dim
I = 4096      # mlp intermediate
NC = 8        # cores
R = S // NC   # rows per core = 256
HT = H // 128  # hidden tiles = 8
KT = S // 128  # key tiles = 16
EPS = 1e-6

F32 = mybir.dt.float32
BF16 = mybir.dt.bfloat16
AF = mybir.ActivationFunctionType
BF16NP = ml_dtypes.bfloat16

_CACHED_MODULE = None


def _split_multi_waits(nc):
    """This env's walrus rejects >1 sem wait per instruction.
    Hoist extra waits onto preceding single-wait NoOps on the same engine."""
    limit = 1
    n_split = 0
    for f in nc.m.functions:
        for bb in f.blocks:
            new_insts = []
            for inst in bb.instructions:
                si = getattr(inst, "sync_info", None)
                if (si is not None and getattr(inst, "engine", None) is not None
                        and len(si.on_wait) > limit):
                    waits = list(si.on_wait)
                    hoist, keep = waits[:-limit], waits[-limit:]
                    for i, w in enumerate(hoist):
                        new_insts.append(
                            mybir.InstNoOp(
                                name=f"{inst.name}_waitsplit_{i}",
                                engine=inst.engine,
                                sync_info=mybir.SyncInfo(on_wait=[w], on_update=[]),
                                bass_nofuse=True,
                            )
                        )
                        n_split += 1
                    si.on_wait = keep
                new_insts.append(inst)
            bb.instructions = new_insts
    return n_split


def build_module():
    global _CACHED_MODULE
    if _CACHED_MODULE is not None:
        return _CACHED_MODULE

    nc = bass.Bass(num_devices=NC)

    # --- kernel I/O (per-core) ---
    xT = nc.declare_dram_parameter("xT", [H, R], F32, isOutput=False)
    xbT = nc.declare_dram_parameter("xbT", [H, R], BF16, isOutput=False)
    ctxT = nc.declare_dram_parameter("ctxT", [H, R], BF16, isOutput=False)
    wnames = ["sa_wq", "sa_wk", "sa_wv", "sa_wo", "ca_wq", "ca_wk", "ca_wv", "ca_wo"]
    W = {n: nc.declare_dram_parameter(n, [H, H], BF16, isOutput=False) for n in wnames}
    W["w_gate"] = nc.declare_dram_parameter("w_gate", [H, I], BF16, isOutput=False)
    W["w_up"] = nc.declare_dram_parameter("w_up", [H, I], BF16, isOutput=False)
    W["w_down"] = nc.declare_dram_parameter("w_down", [I, H], BF16, isOutput=False)
    outT = nc.declare_dram_parameter("outT", [H, R], F32, isOutput=True)

    groups = [list(range(NC))]

    with tile.TileContext(nc) as tc:
        with (
            tc.tile_pool(name="p1", bufs=1) as p1,        # long-lived singles
            tc.tile_pool(name="p2", bufs=2) as p2,        # rotating pairs
            tc.tile_pool(name="resid", bufs=2) as presid, # xT / h1 / h2 fp32
            tc.tile_pool(name="wts", bufs=4) as pw,       # 2MB weight slabs
            tc.tile_pool(name="psA", bufs=5, space="PSUM") as psA,
            tc.tile_pool(name="psB", bufs=3, space="PSUM") as psB,
            tc.tile_pool(name="dram", bufs=1, space="DRAM") as pdram,
        ):
            # --- constants ---
            inv_h = p1.tile([128, 1], BF16, tag="inv_h")
            nc.vector.memset(inv_h[:], 1.0 / H)
            eps_c = p1.tile([1, 1], F32, tag="eps_c")
            nc.vector.memset(eps_c[:], EPS)
            # ones row for K=1 outer-product broadcasts
            ones_row = p1.tile([1, 128], BF16, tag="ones_row")
            nc.vector.memset(ones_row[:], 1.0)
            # --- input loads: xb/xt FIRST on sync (ahead of the weight stream —
            # they gate the whole front of the schedule); ctx on scalar (needed
            # ~40us later, rides the leftover bandwidth) ---
            xb_sb = p1.tile([128, HT, R], BF16, tag="xb_sb")
            nc.sync.dma_start(xb_sb[:], xbT.rearrange("(t p) q -> p t q", p=128))
            xt_sb = presid.tile([128, HT, R], F32, tag="resid", name="xt_sb")
            nc.sync.dma_start(xt_sb[:], xT.rearrange("(t p) q -> p t q", p=128))
            ctx_sb = p1.tile([128, HT, R], BF16, tag="ctx_sb")
            nc.sync.dma_start(ctx_sb[:], ctxT.rearrange("(t p) q -> p t q", p=128))

            def load_w(dram_t, cols=None, rows=None, name="w"):
                """Load a [1024, M<=1024] slab of a weight matrix as [128, 8, M] bf16
                on the sync queue (weights only -> continuous prefetch stream)."""
                ap = dram_t.rearrange("(t p) m -> p t m", p=128)
                if rows is not None:  # row-chunk of a tall matrix (w_down)
                    ap = dram_t[rows[0]:rows[1], :].rearrange("(t p) m -> p t m", p=128)
                if cols is not None:
                    ap = ap[:, :, cols[0]:cols[1]]
                m = ap.shape[2]
                t = pw.tile([128, HT, 1024], BF16, tag="w", name=name)
                nc.sync.dma_start(t[:, :, :m], ap)
                return t

            def rstd_broadcast(src_f32, name, pool=None):
                """1/rms of each column of src, broadcast to a [128, R] bf16 SBUF
                tile (partition-broadcast via K=1 ones outer-product matmul)."""
                pool = pool or psA
                var = pool.tile([128, 512], F32, tag=pool is psA and "psA" or "psB",
                                name=f"{name}_var")
                for t in range(HT):
                    sq = p2.tile([128, R], BF16, tag="sq", name=f"{name}_sq{t}")
                    nc.scalar.activation(sq[:], src_f32[:, t, :], AF.Square)
                    nc.tensor.matmul(var[:1, :R], inv_h[:], sq[:],
                                     start=(t == 0), stop=(t == HT - 1))
                sd = p2.tile([1, R], F32, tag="sd", name=f"{name}_sd")
                nc.scalar.activation(sd[:], var[:1, :R], AF.Sqrt, bias=eps_c[:])
                rstd = p2.tile([1, R], BF16, tag="rstd", name=f"{name}_rstd")
                with nc.allow_low_precision(reason="bf16 rstd broadcast, tol 2e-2"):
                    nc.vector.reciprocal(rstd[:], sd[:])
                rb = pool.tile([128, 512], F32, tag=pool is psA and "psA" or "psB",
                               name=f"{name}_rb")
                # both 256-halves get the same broadcast so 512-wide consumers
                # (m-paired psum tiles) can multiply in one op
                nc.tensor.matmul(rb[:, :R], ones_row[:1, :], rstd[:1, :],
                                 start=True, stop=False)
                nc.tensor.matmul(rb[:, R:2 * R], ones_row[:1, :], rstd[:1, :],
                                 start=False, stop=True, skip_group_check=True)
                rbs = p1.tile([128, 2 * R], BF16, tag=f"rbs_{name}", name=f"{name}_rbs")
                nc.vector.tensor_copy(rbs[:], rb[:, :2 * R])
                return rbs

            def mm_pair(ps, w_sb, mp, act_sb):
                """16-matmul contiguous accumulation: output tiles 2mp/2mp+1
                side by side in one psum bank.  start=True on the first clears
                the whole bank; the first write to each 256-col half lands on
                has_written=0 so accumulates from zero.  Long contiguous groups
                keep the PE p-state ramped."""
                for t in range(HT):
                    for half in range(2):
                        m = 2 * mp + half
                        nc.tensor.matmul(
                            ps[:, R * half:R * (half + 1)],
                            w_sb[:, t, 128 * m:128 * (m + 1)], act_sb[:, t, :],
                            start=(t == 0 and half == 0),
                            stop=(t == HT - 1 and half == 1),
                            skip_group_check=True)

            def mm_quad(specs, act_sb):
                """Two m-pair accumulations (separate psum banks) interleaved
                into ONE 32-matmul PE stream with a single start/stop per bank.
                32-long uninterrupted streams are what actually reach the
                2.4GHz p-state (16-long groups stay at 1.2GHz).
                specs: [(ps, w_sb, mp), (ps2, w_sb2, mp2)]."""
                for t in range(HT):
                    for half in range(2):
                        for ps, w_sb, mp in specs:
                            m = 2 * mp + half
                            nc.tensor.matmul(
                                ps[:, R * half:R * (half + 1)],
                                w_sb[:, t, 128 * m:128 * (m + 1)], act_sb[:, t, :],
                                start=(t == 0 and half == 0),
                                stop=(t == HT - 1 and half == 1),
                                skip_group_check=True)

            def proj(w_sb, act_sb, dst_bf16, rbs=None):
                """dst[*, m, :] (bf16 [128, HT, R]) = (W.T @ act) [* rbs column scale]."""
                for mq in range(HT // 4):
                    pss = [psA.tile([128, 512], F32, tag="psA", name=f"pj_{2 * mq + i}")
                           for i in range(2)]
                    mm_quad([(pss[0], w_sb, 2 * mq), (pss[1], w_sb, 2 * mq + 1)], act_sb)
                    for i in range(2):
                        mp = 2 * mq + i
                        dst2 = dst_bf16[:, 2 * mp:2 * mp + 2, :].rearrange("p a b -> p (a b)")
                        if rbs is None:
                            nc.vector.tensor_copy(dst2, pss[i][:, :2 * R])
                        else:
                            nc.vector.tensor_mul(dst2, pss[i][:, :2 * R], rbs[:])

            def proj_add(w_sb, act_sb, resid_f32, dst_f32):
                """dst (f32 [128, HT, R]) = resid + W.T @ act."""
                for mq in range(HT // 4):
                    pss = [psA.tile([128, 512], F32, tag="psA", name=f"pa_{2 * mq + i}")
                           for i in range(2)]
                    mm_quad([(pss[0], w_sb, 2 * mq), (pss[1], w_sb, 2 * mq + 1)], act_sb)
                    for i in range(2):
                        mp = 2 * mq + i
                        nc.vector.tensor_add(
                            dst_f32[:, 2 * mp:2 * mp + 2, :].rearrange("p a b -> p (a b)"),
                            pss[i][:, :2 * R],
                            resid_f32[:, 2 * mp:2 * mp + 2, :].rearrange("p a b -> p (a b)"))

            def kv_block(wk_sb, wv_sb, act_k, act_v_fn, blk,
                         rstd_src=None, rstd_name=None):
                """Compute own-row K^T [1024, R] and V [R, 1024] (bf16), stage to
                DRAM (scalar queue) and AllGather (gpsimd).  K runs on act_k (raw
                activation); if rstd_src is given, the rmsnorm rstd is computed
                CONCURRENTLY (var/rb on psB, interleaved after K group 3 so the
                PE starts on K immediately) and folded into the PSUM copies.
                act_v_fn(rbs) supplies the V activation (pre-normed; V's gather
                has slack behind K's).  Returns (k_out, v_out, rbs)."""
                k_in = pdram.tile([H * R], BF16, tag=f"kin{blk}")
                k_out = pdram.tile([NC, H * R], BF16, tag=f"kout{blk}",
                                   addr_space="Shared")
                v_in = pdram.tile([H * R], BF16, tag=f"vin{blk}")
                v_out = pdram.tile([NC, H * R], BF16, tag=f"vout{blk}",
                                   addr_space="Shared")
                k_view = k_in.rearrange("(t p q) -> p t q", t=HT, p=128, q=R)
                v_view = v_in.rearrange("(mt p d) -> p mt d", mt=2, p=128, d=1024)
                rbs = None
                pending = []

                def emit_k_copy(mp, ps):
                    stg = p2.tile([128, 512], BF16, tag="stg", bufs=2,
                                  name=f"ks{blk}_{mp}")
                    if rbs is None:
                        nc.vector.tensor_copy(stg[:, :2 * R], ps[:, :2 * R])
                    else:
                        nc.vector.tensor_mul(stg[:, :2 * R], ps[:, :2 * R], rbs[:])
                    nc.scalar.dma_start(
                        k_view[:, 2 * mp:2 * mp + 2, :],
                        stg[:, :2 * R].rearrange("p (a b) -> p a b", a=2))

                for mq in range(HT // 4):
                    pss = [psA.tile([128, 512], F32, tag="psA", name=f"k{blk}_{2 * mq + i}")
                           for i in range(2)]
                    mm_quad([(pss[0], wk_sb, 2 * mq), (pss[1], wk_sb, 2 * mq + 1)], act_k)
                    if rstd_src is not None and rbs is None:
                        pending += [(2 * mq, pss[0]), (2 * mq + 1, pss[1])]
                        if mq == 0:
                            rbs = rstd_broadcast(rstd_src, rstd_name, pool=psB)
                            for mmp, pps in pending:
                                emit_k_copy(mmp, pps)
                            pending = []
                    else:
                        for i in range(2):
                            emit_k_copy(2 * mq + i, pss[i])
                nc.gpsimd.collective_compute(
                    "AllGather", mybir.AluOpType.bypass, replica_groups=groups,
                    ins=[k_in[:]], outs=[k_out[:]])
                act_v = act_v_fn(rbs)
                for mt in range(2):
                    for nchunk in range(2):
                        ps = psA.tile([128, 512], F32, tag="psA", name=f"v{blk}_{mt}_{nchunk}")
                        for t in range(HT):
                            nc.tensor.matmul(
                                ps[:], act_v[:, t, 128 * mt:128 * (mt + 1)],
                                wv_sb[:, t, 512 * nchunk:512 * (nchunk + 1)],
                                start=(t == 0), stop=(t == HT - 1))
                        stg = p2.tile([128, 512], BF16, tag="stg", bufs=2,
                                      name=f"vs{blk}_{mt}_{nchunk}")
                        nc.vector.tensor_copy(stg[:], ps[:])
                        nc.scalar.dma_start(v_view[:, mt, 512 * nchunk:512 * (nchunk + 1)], stg[:])
                nc.gpsimd.collective_compute(
                    "AllGather", mybir.AluOpType.bypass, replica_groups=groups,
                    ins=[v_in[:]], outs=[v_out[:]])
                return k_out, v_out, rbs

            def attention(q_sb, k_out, v_out, blk):
                """q_sb [128, HT, R] bf16 (feature-major, all heads), k_out/v_out from
                kv_block. Returns attnT [128, HT, R] bf16 = softmax(qk)V transposed."""
                vsb = p1.tile([128, KT, NH, HD + 1], BF16, tag="vsb", name=f"vsb{blk}")
                attnT = p1.tile([128, HT, R], BF16, tag="attnT", name=f"attnT{blk}")
                nc.vector.memset(vsb[:, :, :, HD:HD + 1], 1.0)

                def load_vsb():
                    # sync queue: gated on the V AllGather.  On scalar it would
                    # head-of-line block the kf/EXP stream; on sync it only
                    # pauses weight prefetch, which has slack here.  Emitted at
                    # the END of dt==0 so every av_pair (first at dt==1) is
                    # emission-ordered after these writes — Tile only orders
                    # reads after writes already emitted.  Two batched triggers
                    # (ranks 0-3 / 4-7) instead of 16 per-kt ones.
                    for kt in range(KT):
                        r, mt = kt // 2, kt % 2
                        src = v_out[r].rearrange(
                            "(mt p hd d) -> p mt hd d", mt=2, p=128, hd=NH, d=HD)
                        nc.sync.dma_start(vsb[:, kt, :, 0:HD], src[:, mt, :, :])

                pts_all = []
                for dt in range(HT):  # head pair dt = heads 2dt, 2dt+1
                    kf = p2.tile([128, NC, R], BF16, tag="kf", bufs=2, name=f"kf{blk}_{dt}")
                    # ONE batched trigger for all 8 ranks, on the SYNC queue:
                    # the scalar sequencer must stay pure EXP during attention
                    # (EXP throughput is the phase's floor).
                    nc.sync.dma_start(
                        kf[:, :, :],
                        k_out.rearrange("r (t p q) -> p t r q", t=HT, p=128, q=R)[:, dt, :, :])
                    # Both heads' score matmuls are emitted ADJACENTLY so the
                    # (0,0)/(64,0) tile_position row-packing runs them
                    # concurrently on the PE array.
                    # bufs=4: the lag-1 AV keeps pair dt-1 alive while pair dt's
                    # EXPs write.  MUST be >= 4 — with 3, pt(dt,1) would steal
                    # pt(dt-1,0)'s slot before its AV reads are even emitted.
                    pts = [p2.tile([128, KT, R], BF16, tag="pt", bufs=4,
                                   name=f"pt{blk}_{2 * dt + hh}") for hh in range(2)]
                    for kp in range(KT // 2):
                        spss = [psA.tile([128, 512], F32, tag="psA",
                                         name=f"s{blk}_{2 * dt + hh}_{kp}") for hh in range(2)]
                        for j in range(2):
                            kt = 2 * kp + j
                            r2, mt2 = kt // 2, kt % 2
                            for hh in range(2):
                                off = HD * hh
                                # one start/stop per psum tile (j halves share a
                                # group) — per-matmul start/stop resets the PE
                                # p-state ramp
                                nc.tensor.matmul(
                                    spss[hh][:, R * j:R * (j + 1)],
                                    kf[off:off + HD, r2, 128 * mt2:128 * (mt2 + 1)],
                                    q_sb[off:off + HD, dt, :],
                                    start=(j == 0), stop=(j == 1),
                                    tile_position=(off, 0), skip_group_check=True)
                        for hh in range(2):
                            # flat 2D APs (the pt slice is contiguous): fewer
                            # AP dims = less ACT-engine per-instruction setup,
                            # and EXP throughput is the attention floor
                            nc.scalar.activation(
                                pts[hh][:, 2 * kp:2 * kp + 2, :]
                                .rearrange("p a b -> p (a b)"),
                                spss[hh][:, :2 * R], AF.Exp)
                    pts_all.append(pts)
                    if dt == 0:
                        load_vsb()

                    # AV for head pair dt-1 (lag so scores of dt overlap the
                    # vsb wait / epilogue of earlier heads)
                    if dt >= 1:
                        av_pair(vsb, attnT, pts_all[dt - 1], dt - 1, blk)
                av_pair(vsb, attnT, pts_all[HT - 1], HT - 1, blk)
                flush_epilogue(attnT, blk)
                return attnT

            # --- AV + epilogue with software pipelining (module-level state) ---
            epi_state = []

            def av_pair(vsb, attnT, pts, dt, blk):
                """Both heads of pair dt in ONE 32-matmul psum group (heads side
                by side in the bank's 256-col halves) — long contiguous PE runs
                keep the p-state ramped."""
                avps = psB.tile([128, 512], F32, tag="psB", name=f"av{blk}_{dt}")
                for kt in range(KT):
                    for hh in range(2):
                        nc.tensor.matmul(
                            avps[:HD + 1, R * hh:R * (hh + 1)],
                            vsb[:, kt, 2 * dt + hh, :], pts[hh][:, kt, :],
                            start=(kt == 0 and hh == 0),
                            stop=(kt == KT - 1 and hh == 1),
                            skip_group_check=True)
                # vector part of the pair's epilogue: one 512-wide recip + copy
                rl = p2.tile([1, 2 * R], BF16, tag="rl", name=f"rl{blk}_{dt}")
                with nc.allow_low_precision(reason="bf16 1/l broadcast, tol 2e-2"):
                    nc.vector.reciprocal(rl[:], avps[HD:HD + 1, :2 * R])
                av_sb = p2.tile([HD, 2 * R], BF16, tag="av_sb", bufs=2,
                                name=f"avs{blk}_{dt}")
                nc.vector.tensor_copy(av_sb[:], avps[0:HD, :2 * R])
                epi_state.append((rl, av_sb, dt))
                # PE part (rlb broadcast matmuls) of the OLDEST pending pair:
                # one pair behind, so its wait on the DVE reciprocal is hidden
                # behind this pair's AV stream.
                if len(epi_state) > 1:
                    emit_rlb(attnT, blk, epi_state.pop(0))

            def emit_rlb(attnT, blk, st):
                rl, av_sb, dt = st
                rlb = psB.tile([128, 512], F32, tag="psB", name=f"rlb{blk}_{dt}")
                nc.tensor.matmul(rlb[:HD, :R], ones_row[:1, :HD], rl[:1, :R],
                                 start=True, stop=False)
                nc.tensor.matmul(rlb[:HD, R:2 * R], ones_row[:1, :HD], rl[:1, R:2 * R],
                                 start=False, stop=True, skip_group_check=True)
                for hh in range(2):
                    off = HD * hh
                    nc.vector.tensor_mul(attnT[off:off + HD, dt, :],
                                         av_sb[:, R * hh:R * (hh + 1)],
                                         rlb[:HD, R * hh:R * (hh + 1)])

            def flush_epilogue(attnT, blk):
                while epi_state:
                    emit_rlb(attnT, blk, epi_state.pop(0))

            # ---------------- self-attention block ----------------
            wk_sb = load_w(W["sa_wk"], name="sa_wk_sb")
            wv_sb = load_w(W["sa_wv"], name="sa_wv_sb")

            def make_xn(rbs):
                # normed x for the V projection
                xn = p1.tile([128, HT, R], BF16, tag="normed", name="xn")
                for t in range(HT):
                    nc.vector.tensor_mul(xn[:, t, :], xb_sb[:, t, :], rbs[:, :R])
                return xn

            k1, v1, rbs0 = kv_block(wk_sb, wv_sb, xb_sb, make_xn, 0,
                                    rstd_src=xt_sb, rstd_name="n0")

            # cross-attn K/V depend only on raw context: compute + AG them early
            # so both gathers overlap the self-attention phase on the CC engine.
            wk2_sb = load_w(W["ca_wk"], name="ca_wk_sb")
            wv2_sb = load_w(W["ca_wv"], name="ca_wv_sb")
            k2, v2, _ = kv_block(wk2_sb, wv2_sb, ctx_sb, lambda rbs: ctx_sb, 1)

            # Weight-load EMISSION order is chosen so each pw-slot reuse is
            # emitted after the previous occupant's readers (Tile only orders
            # against already-emitted reads), and so every slab's runtime gate
            # (slot free) opens just before the data is needed:
            #   slabs 1-8: wk,wv,wk2,wv2,wq | wo,wq2,wo2   9-12: wg0,wu0 wg1,wu1
            #   13-16: wg2,wu2,wg3,wu3 (emitted inside the MLP loop)
            #   17-20: wd0-3 (emitted inside the MLP loop)
            wq_sb = load_w(W["sa_wq"], name="sa_wq_sb")
            wo_sb = load_w(W["sa_wo"], name="sa_wo_sb")
            wq2_sb = load_w(W["ca_wq"], name="ca_wq_sb")
            wo2_sb = load_w(W["ca_wo"], name="ca_wo_sb")
            qT = p1.tile([128, HT, R], BF16, tag="qt", name="qT")
            proj(wq_sb, xb_sb, qT, rbs=rbs0)
            wgu = [load_w(W["w_gate"], cols=(0, 1024), name="wg0")]

            attnT = attention(qT, k1, v1, 0)
            h1 = presid.tile([128, HT, R], F32, tag="resid", name="h1")
            proj_add(wo_sb, attnT, xt_sb, h1)
            wgu.append(load_w(W["w_up"], cols=(0, 1024), name="wu0"))

            # ---------------- cross-attention block ----------------
            rbs1 = rstd_broadcast(h1, "n1")
            h1b = p1.tile([128, HT, R], BF16, tag="normed", name="h1b")
            for t in range(HT):
                nc.vector.tensor_copy(h1b[:, t, :], h1[:, t, :])
            qT2 = p1.tile([128, HT, R], BF16, tag="qt", name="qT2")
            proj(wq2_sb, h1b, qT2, rbs=rbs1)
            wgu.append(load_w(W["w_gate"], cols=(1024, 2048), name="wg1"))

            attnT2 = attention(qT2, k2, v2, 1)
            h2 = presid.tile([128, HT, R], F32, tag="resid", name="h2")
            proj_add(wo2_sb, attnT2, h1, h2)
            wgu.append(load_w(W["w_up"], cols=(1024, 2048), name="wu1"))

            # ---------------- MLP block ----------------
            # NOTE: start=True clears has_written for the WHOLE psum bank, so each
            # accumulation group must own its bank exclusively for its entire
            # lifetime.  Phase A computes all 32 act subtiles into SBUF; phase B
            # runs one contiguous 32-matmul accumulation per output tile.
            rbs2 = rstd_broadcast(h2, "n2")
            hn2 = p1.tile([128, HT, R], BF16, tag="normed", name="hn2")
            for t in range(HT):
                nc.vector.tensor_mul(hn2[:, t, :], h2[:, t, :], rbs2[:, :R])
            NCHUNK = 4  # I-chunks of 1024
            # reuses xb_sb's slot (its readers — K1/q matmuls — are long done)
            act_full = p1.tile([128, I // 128, R], BF16, tag="xb_sb")  # 2MB
            wds = []
            for c in range(NCHUNK):
                wg_sb, wu_sb = wgu[2 * c], wgu[2 * c + 1]
                for mip in range(4):
                    gps = psA.tile([128, 512], F32, tag="psA", name=f"g{c}_{mip}")
                    ups = psA.tile([128, 512], F32, tag="psA", name=f"u{c}_{mip}")
                    mm_quad([(gps, wg_sb, mip), (ups, wu_sb, mip)], hn2)
                    gsil = p2.tile([128, 2 * R], BF16, tag="gsil", name=f"gs{c}_{mip}")
                    nc.scalar.activation(gsil[:], gps[:, :2 * R], AF.Silu)
                    nc.vector.tensor_mul(
                        act_full[:, 8 * c + 2 * mip:8 * c + 2 * mip + 2, :]
                        .rearrange("p a b -> p (a b)"),
                        ups[:, :2 * R], gsil[:])
                # prefetch: next gate/up pair, then the down slabs
                if c < NCHUNK - 2:
                    cc = c + 2
                    wgu.append(load_w(W["w_gate"], cols=(1024 * cc, 1024 * (cc + 1)), name=f"wg{cc}"))
                    wgu.append(load_w(W["w_up"], cols=(1024 * cc, 1024 * (cc + 1)), name=f"wu{cc}"))
                elif c == NCHUNK - 2:
                    wds.append(load_w(W["w_down"], rows=(0, 1024), name="wd0"))
                    wds.append(load_w(W["w_down"], rows=(1024, 2048), name="wd1"))
                else:
                    wds.append(load_w(W["w_down"], rows=(2048, 3072), name="wd2"))
                    wds.append(load_w(W["w_down"], rows=(3072, 4096), name="wd3"))
            out_sb = p1.tile([128, HT, R], F32, tag="out_sb")
            for m in range(HT):
                dps = psB.tile([128, 512], F32, tag="psB", name=f"dp{m}")
                for s in range(I // 128):
                    nc.tensor.matmul(dps[:, :R], wds[s // 8][:, s % 8, 128 * m:128 * (m + 1)],
                                     act_full[:, s, :],
                                     start=(s == 0), stop=(s == I // 128 - 1))
                nc.vector.tensor_add(out_sb[:, m, :], dps[:, :R], h2[:, m, :])
            nc.sync.dma_start(outT.rearrange("(t p) q -> p t q", p=128), out_sb[:])

    _split_multi_waits(nc)
    _CACHED_MODULE = nc
    return nc


def prep_in_maps(hidden_states, context, sa_norm_w, sa_wq, sa_wk, sa_wv, sa_wo,
                 ca_norm_w, ca_wq, ca_wk, ca_wv, ca_wo,
                 mlp_norm_w, w_gate, w_up, w_down):
    f32 = np.float32
    x = np.asarray(hidden_states, f32).reshape(S, H)
    ctx = np.asarray(context, f32).reshape(S, H)
    xT_full = np.ascontiguousarray(x.T)                      # [H, S] f32
    xbT_full = xT_full.astype(BF16NP)                        # [H, S] bf16
    ctxT_full = np.ascontiguousarray(ctx.T).astype(BF16NP)   # [H, S] bf16

    def bf(a):
        return np.ascontiguousarray(np.asarray(a, f32)).astype(BF16NP)

    sa_w = np.asarray(sa_norm_w, f32)[:, None]
    ca_w = np.asarray(ca_norm_w, f32)[:, None]
    mlp_w = np.asarray(mlp_norm_w, f32)[:, None]
    scale = HD ** -0.5
    shared = {
        "sa_wq": bf(sa_w * np.asarray(sa_wq, f32) * scale),
        "sa_wk": bf(sa_w * np.asarray(sa_wk, f32)),
        "sa_wv": bf(sa_w * np.asarray(sa_wv, f32)),
        "sa_wo": bf(sa_wo),
        "ca_wq": bf(ca_w * np.asarray(ca_wq, f32) * scale),
        "ca_wk": bf(ca_wk),
        "ca_wv": bf(ca_wv),
        "ca_wo": bf(ca_wo),
        "w_gate": bf(mlp_w * np.asarray(w_gate, f32)),
        "w_up": bf(mlp_w * np.asarray(w_up, f32)),
        "w_down": bf(w_down),
    }
    in_maps = []
    for r in range(NC):
        m = dict(shared)
        m["xT"] = np.ascontiguousarray(xT_full[:, r * R:(r + 1) * R])
        m["xbT"] = np.ascontiguousarray(xbT_full[:, r * R:(r + 1) * R])
        m["ctxT"] = np.ascontiguousarray(ctxT_full[:, r * R:(r + 1) * R])
        in_maps.append(m)
    return in_maps


def run_spmd(in_maps, **kwargs):
    from concourse.bass_utils import run_bass_kernel_spmd
    nc = build_module()
    return run_bass_kernel_spmd(nc, in_maps, core_ids=list(range(NC)), **kwargs)


def kernel(**inputs):
    in_maps = prep_in_maps(**inputs)
    res = run_spmd(in_maps)
    out = np.empty((1, S, H), np.float32)
    for r in range(NC):
        out[0, r * R:(r + 1) * R, :] = res.results[r]["outT"].T
    return out
